# revision 5
# baseline (speedup 1.0000x reference)
"""Trainium2 Bass kernel for a pre-norm causal-attention transformer layer.

Contract: kernel(**inputs) takes the FULL fp32 inputs of reference.setup_inputs()
and returns the FULL (1, 4096, 1024) fp32 output, distributing across 8
NeuronCores internally (heads tensor-parallel for attention, tokens
data-parallel for the output projection + FFN, one AllToAll in between).

Math notes (validated against the reference in fp64/numpy):
- LayerNorm gains are folded into the following weight matrices on the host:
  h @ W = z @ (g*W) + (ln_b @ W), where z = (x - mu) * rsig.
- The k-projection bias is dropped (softmax is shift-invariant along keys);
  the v bias is applied after normalization; the q bias rides the eviction.
- Softmax runs without max-subtraction (scores are bounded, |s| < ~3).
- Scores are built transposed (keys on partitions) so exp output feeds the
  PE directly; an appended ones-column of v yields the denominator row.
"""

import sys

sys.path.insert(0, "/opt/trn_rl_repo")

import ml_dtypes
import numpy as np

import concourse.bass as bass
from concourse import bacc, mybir, tile
from concourse.bass_utils import run_bass_kernel_spmd

F32 = mybir.dt.float32
BF = mybir.dt.bfloat16
bf16 = ml_dtypes.bfloat16

P = 128
E = 1024
NH = 16
HS = 64
D = 1024
FF = 4096
NCORES = 8
HPC = NH // NCORES  # heads per core = 2
LN_EPS = 1e-5
SCL = 1.0 / 32.0  # 1/sqrt(E)

Act = mybir.ActivationFunctionType
Alu = mybir.AluOpType


def _build(C):
    NT = C // P  # x tiles (32)
    NQC = C // 512  # q chunks (8)
    TPC = C // NCORES  # tokens per core (512)
    TT = TPC // P  # token tiles per core slice (4)
    NZG = max(1, NT // 8)  # zT groups of 8 x-tiles
    GL = NT // NZG  # x-tiles per zT group
    KT_E = E // P  # contraction tiles over E (8)
    KT_F = FF // P  # contraction tiles over FF (32)
    NFT = FF // P  # f tiles (32)

    nc = bacc.Bacc("TRN2", target_bir_lowering=False, debug=False, num_devices=NCORES)

    x_d = nc.dram_tensor("x", [C, E], F32, kind="ExternalInput")
    xs_d = nc.dram_tensor("xs", [TPC, E], F32, kind="ExternalInput")
    wq_d = nc.dram_tensor("wq", [KT_E, P, P], BF, kind="ExternalInput")
    wk_d = nc.dram_tensor("wk", [KT_E, P, P], BF, kind="ExternalInput")
    wv_d = nc.dram_tensor("wv", [KT_E, P, P], BF, kind="ExternalInput")
    bq_d = nc.dram_tensor("bq", [P, 1], F32, kind="ExternalInput")
    bv_d = nc.dram_tensor("bv", [P, 1], F32, kind="ExternalInput")
    wo_d = nc.dram_tensor("wo", [KT_E, P, D], BF, kind="ExternalInput")
    bo_d = nc.dram_tensor("bo_r", [1, D], F32, kind="ExternalInput")
    w1_d = nc.dram_tensor("w1", [KT_E, P, FF], BF, kind="ExternalInput")
    b1_d = nc.dram_tensor("b1c", [P, NFT], F32, kind="ExternalInput")
    w2_d = nc.dram_tensor("w2", [KT_F, P, E], BF, kind="ExternalInput")
    b2_d = nc.dram_tensor("b2_r", [1, E], F32, kind="ExternalInput")
    mask_d = nc.dram_tensor("mask", [P, P], BF, kind="ExternalInput")
    y_d = nc.dram_tensor("y", [TPC, E], F32, kind="ExternalOutput")
    y_view = y_d.ap().rearrange("(tc p) e -> p tc e", p=P)
    xs_view = xs_d.ap().rearrange("(tc p) e -> p tc e", p=P)

    with tile.TileContext(nc) as tc:
        with (
            tc.tile_pool(name="consts", bufs=1) as consts,
            tc.tile_pool(name="dram", bufs=1, space="DRAM") as dram,
        ):
            # ---- constants / weights resident in SBUF (~23 KB/part) ----
            wq_sb = consts.tile([P, KT_E, P], BF, tag="wq")
            wk_sb = consts.tile([P, KT_E, P], BF, tag="wk")
            wv_sb = consts.tile([P, KT_E, P], BF, tag="wv")
            nc.sync.dma_start(wq_sb, wq_d.ap().rearrange("k p m -> p k m"))
            nc.sync.dma_start(wk_sb, wk_d.ap().rearrange("k p m -> p k m"))
            nc.sync.dma_start(wv_sb, wv_d.ap().rearrange("k p m -> p k m"))
            wo_sb = consts.tile([P, KT_E, D], BF, tag="wo")
            nc.sync.dma_start(wo_sb, wo_d.ap().rearrange("k p n -> p k n"))
            bq_sb = consts.tile([P, 1], F32, tag="bq")
            bv_sb = consts.tile([P, 1], F32, tag="bv")
            nc.sync.dma_start(bq_sb, bq_d.ap())
            nc.sync.dma_start(bv_sb, bv_d.ap())
            bo_sb = consts.tile([1, D], F32, tag="bo")
            b2_sb = consts.tile([1, E], F32, tag="b2")
            nc.sync.dma_start(bo_sb, bo_d.ap())
            nc.sync.dma_start(b2_sb, b2_d.ap())
            b1_sb = consts.tile([P, NFT], F32, tag="b1")
            nc.sync.dma_start(b1_sb, b1_d.ap())
            mask_sb = consts.tile([P, P], BF, tag="mask")
            nc.sync.dma_start(mask_sb, mask_d.ap())
            eps_sb = consts.tile([P, 1], F32, tag="eps")
            nc.vector.memset(eps_sb, LN_EPS)
            ones_sb = consts.tile([1, P], F32, tag="ones")
            nc.vector.memset(ones_sb, 1.0)

            a2a_in = dram.tile([NCORES, P, TPC], BF, tag="a2a_in")
            a2a_out = dram.tile([NCORES, P, TPC], BF, tag="a2a_out")

            # ======== attention scope: qT/kT/v/outT (~32 KB/part) ========
            with tc.tile_pool(name="attnb", bufs=1) as attnb:
                qT_sb = attnb.tile([P, C], BF, tag="qT")  # [2 heads x 64, C]
                kT_sb = attnb.tile([P, C], BF, tag="kT")
                v_sb = attnb.tile([P, NT, HPC, HS + 1], BF, tag="v")  # [v_h|1]
                outT_sb = attnb.tile([P, C], BF, tag="outT")
                nc.vector.memset(v_sb[:, :, :, HS : HS + 1], 1.0)

                # ---- phase 1: LN1 + transpose (z with E on partitions) ----
                with (
                    tc.tile_pool(name="xp", bufs=3) as xp,
                    tc.tile_pool(name="zp", bufs=3) as zp,
                    tc.tile_pool(name="stp", bufs=3) as stp,
                    tc.tile_pool(name="ztp", bufs=1) as ztp,
                    tc.tile_pool(name="qkps", bufs=2, space="PSUM") as qkps,
                    tc.tile_pool(name="vps", bufs=2, space="PSUM") as vps,
                ):
                    zT_g = [
                        ztp.tile([P, GL, KT_E, P], BF, name=f"zT{g}", tag=f"zT{g}")
                        for g in range(NZG)
                    ]
                    for t in range(NT):
                        x_sb = xp.tile([P, E], F32, tag="xt")
                        nc.sync.dma_start(x_sb, x_d[t * P : (t + 1) * P, :])
                        st = stp.tile([P, 2, 6], F32, tag="st")
                        nc.vector.bn_stats(st[:, 0, :], x_sb[:, 0:512])
                        nc.vector.bn_stats(st[:, 1, :], x_sb[:, 512:1024])
                        mv = stp.tile([P, 2], F32, tag="mv")
                        nc.vector.bn_aggr(mv, st)
                        sig = stp.tile([P, 1], F32, tag="sig")
                        nc.scalar.activation(
                            sig, mv[:, 1:2], Act.Sqrt, bias=eps_sb, scale=1.0
                        )
                        rsig = stp.tile([P, 1], F32, tag="rsig")
                        nc.vector.reciprocal(rsig, sig)
                        negb = stp.tile([P, 1], F32, tag="negb")
                        nc.vector.tensor_scalar(
                            negb, mv[:, 0:1], rsig, -1.0, Alu.mult, Alu.mult
                        )
                        z_sb = zp.tile([P, E], BF, tag="zt")
                        nc.scalar.activation(
                            z_sb, x_sb, Act.Identity, bias=negb, scale=rsig
                        )
                        nc.sync.dma_start(
                            zT_g[t // GL][:, t % GL, :, :], z_sb, transpose=True
                        )

                    # ---- phase 2: q/k/v projections (2 heads stacked, M=128) ----
                    for c in range(C // 512):
                        g, cl = (c * 4) // GL, (c * 4) % GL
                        rhs = zT_g[g][:, cl : cl + 4, :, :]
                        for nm, w, dst in (("q", wq_sb, qT_sb), ("k", wk_sb, kT_sb)):
                            ps = qkps.tile([P, 512], F32, tag="qk")
                            for kt in range(KT_E):
                                nc.tensor.matmul(
                                    ps, w[:, kt, :], rhs[:, :, kt, :],
                                    start=(kt == 0), stop=(kt == KT_E - 1),
                                )
                            if nm == "q":
                                nc.scalar.activation(
                                    dst[:, c * 512 : (c + 1) * 512], ps, Act.Identity,
                                    bias=bq_sb, scale=1.0,
                                )
                            else:
                                nc.scalar.copy(dst[:, c * 512 : (c + 1) * 512], ps)
                    for t in range(NT):
                        ps = vps.tile([P, P], F32, tag="vt")
                        for kt in range(KT_E):
                            nc.tensor.matmul(
                                ps, zT_g[t // GL][:, t % GL, kt, :], wv_sb[:, kt, :],
                                start=(kt == 0), stop=(kt == KT_E - 1),
                            )
                        nc.scalar.copy(v_sb[:, t, 0, 0:HS], ps[:, 0:HS])
                        nc.vector.tensor_copy(v_sb[:, t, 1, 0:HS], ps[:, HS:P])

                # ---- phase 3: causal attention, transposed-score layout ----
                with (
                    tc.tile_pool(name="stps", bufs=3, space="PSUM") as stps,
                    tc.tile_pool(name="avps", bufs=2, space="PSUM") as avps,
                    tc.tile_pool(name="ep", bufs=4) as ep,
                    tc.tile_pool(name="nrm", bufs=3) as nrm,
                    tc.tile_pool(name="recd", bufs=2, space="DRAM") as recd,
                ):
                    for qc in range(NQC):
                        for h in range(HPC):
                            hsl = slice(h * HS, (h + 1) * HS)
                            q_rhs = qT_sb[hsl, qc * 512 : (qc + 1) * 512]
                            av = avps.tile([HS + 1, 512], F32, tag="av")
                            nkb = 4 * qc + 4
                            for kb in range(nkb):
                                sT = stps.tile([P, 512], F32, tag="sT")
                                nc.tensor.matmul(
                                    sT, kT_sb[hsl, kb * P : (kb + 1) * P], q_rhs,
                                    start=True, stop=True,
                                )
                                ex = ep.tile([P, 512], BF, tag="ex")
                                dd = kb - 4 * qc
                                if dd < 0:
                                    nc.scalar.activation(
                                        ex, sT, Act.Exp, bias=0.0, scale=SCL
                                    )
                                else:
                                    if dd > 0:
                                        nc.gpsimd.memset(ex[:, 0 : dd * P], 0.0)
                                    nc.scalar.activation(
                                        ex[:, dd * P : 512], sT[:, dd * P : 512],
                                        Act.Exp, bias=0.0, scale=SCL,
                                    )
                                    nc.vector.tensor_mul(
                                        ex[:, dd * P : (dd + 1) * P],
                                        ex[:, dd * P : (dd + 1) * P], mask_sb,
                                    )
                                nc.tensor.matmul(
                                    av, v_sb[:, kb, h, :], ex,
                                    start=(kb == 0), stop=(kb == nkb - 1),
                                )
                            # normalize rows 0..63 by denominator row 64
                            rec = nrm.tile([1, 512], F32, tag="rec")
                            nc.vector.reciprocal(rec, av[HS : HS + 1, :])
                            rdr = recd.tile([1, 512], F32, tag="rdr")
                            nc.sync.dma_start(rdr, rec)
                            rd = rdr[:]
                            bc = nrm.tile([HS, 512], F32, tag="bc")
                            nc.sync.dma_start(
                                bc,
                                bass.AP(tensor=rd.tensor, offset=rd.offset,
                                        ap=[[0, HS], rd.ap[-1]]),
                            )
                            tmp = nrm.tile([HS, 512], F32, tag="tmp")
                            nc.vector.tensor_mul(tmp, av[0:HS, :], bc)
                            nc.vector.tensor_scalar(
                                outT_sb[hsl, qc * 512 : (qc + 1) * 512], tmp,
                                bv_sb[hsl, 0:1], None, Alu.add,
                            )

                # ship head-outputs: chunk j of outT goes to core j
                nc.sync.dma_start(
                    a2a_in[:].rearrange("j p t -> p j t"),
                    outT_sb.rearrange("p (j t) -> p j t", j=NCORES),
                )

            # ======== FFN scope (attention buffers released) ========
            nc.gpsimd.collective_compute(
                "AllToAll", Alu.bypass,
                replica_groups=[list(range(NCORES))],
                ins=[a2a_in[:].opt()], outs=[a2a_out[:].opt()],
            )
            with tc.tile_pool(name="ffnb", bufs=1) as ffnb:
                oT_sb = ffnb.tile([P, KT_E, TPC], BF, tag="oT")
                nc.sync.dma_start(oT_sb, a2a_out[:].rearrange("j p t -> p j t"))
                x2_sb = ffnb.tile([P, TT, E], F32, tag="x2")
                fT_sb = ffnb.tile([P, NFT, TPC], BF, tag="fT")
                z2T_sb = ffnb.tile([P, TT, KT_E, P], BF, tag="z2T")

                with (
                    tc.tile_pool(name="w1p", bufs=1) as w1p,
                    tc.tile_pool(name="xsp", bufs=2) as xsp,
                    tc.tile_pool(name="st2p", bufs=2) as st2p,
                    tc.tile_pool(name="z2p", bufs=2) as z2p,
                    tc.tile_pool(name="wops", bufs=2, space="PSUM") as wops,
                    tc.tile_pool(name="mm1ps", bufs=3, space="PSUM") as mm1ps,
                ):
                    w1_sb = [
                        w1p.tile([P, FF], BF, name=f"w1_{k}", tag=f"w1_{k}")
                        for k in range(KT_E)
                    ]
                    for kt in range(KT_E):
                        nc.sync.dma_start(w1_sb[kt], w1_d[kt])

                    # ---- phase 4: Wo projection + residual + LN2 ----
                    for t in range(TT):
                        xs_t = xsp.tile([P, E], F32, tag="xst")
                        nc.sync.dma_start(xs_t, xs_view[:, t, :])
                        for n in range(E // 512):
                            ns = slice(n * 512, (n + 1) * 512)
                            ps = wops.tile([P, 512], F32, tag="wo")
                            for kt in range(KT_E):
                                nc.tensor.matmul(
                                    ps, oT_sb[:, kt, t * P : (t + 1) * P],
                                    wo_sb[:, kt, ns],
                                    start=(kt == 0), stop=False,
                                )
                            nc.tensor.matmul(
                                ps, ones_sb, bo_sb[0:1, ns], start=False, stop=True
                            )
                            nc.vector.tensor_add(x2_sb[:, t, ns], ps, xs_t[:, ns])

                        st = st2p.tile([P, 2, 6], F32, tag="st2")
                        nc.vector.bn_stats(st[:, 0, :], x2_sb[:, t, 0:512])
                        nc.vector.bn_stats(st[:, 1, :], x2_sb[:, t, 512:1024])
                        mv = st2p.tile([P, 2], F32, tag="mv2")
                        nc.vector.bn_aggr(mv, st)
                        sig = st2p.tile([P, 1], F32, tag="sig2")
                        nc.scalar.activation(
                            sig, mv[:, 1:2], Act.Sqrt, bias=eps_sb, scale=1.0
                        )
                        rsig = st2p.tile([P, 1], F32, tag="rsig2")
                        nc.vector.reciprocal(rsig, sig)
                        negb = st2p.tile([P, 1], F32, tag="negb2")
                        nc.vector.tensor_scalar(
                            negb, mv[:, 0:1], rsig, -1.0, Alu.mult, Alu.mult
                        )
                        z2 = z2p.tile([P, E], BF, tag="z2")
                        nc.scalar.activation(
                            z2, x2_sb[:, t, :], Act.Identity, bias=negb, scale=rsig
                        )
                        nc.sync.dma_start(z2T_sb[:, t, :, :], z2, transpose=True)

                    # ---- phase 5a: fT = relu(W1.T @ z2T + b1) ----
                    for ft in range(NFT):
                        ps = mm1ps.tile([P, TPC], F32, tag="mm1")
                        for kt in range(KT_E):
                            nc.tensor.matmul(
                                ps, w1_sb[kt][:, ft * P : (ft + 1) * P],
                                z2T_sb[:, :, kt, :],
                                start=(kt == 0), stop=(kt == KT_E - 1),
                            )
                        nc.scalar.activation(
                            fT_sb[:, ft, :], ps, Act.Relu,
                            bias=b1_sb[:, ft : ft + 1], scale=1.0,
                        )

                # ---- phase 5b: y = fT.T @ W2 + b2 + x2 ----
                with (
                    tc.tile_pool(name="mm2ps", bufs=1, space="PSUM") as mm2ps,
                    tc.tile_pool(name="w2p", bufs=3) as w2p,
                    tc.tile_pool(name="yout", bufs=3) as yout,
                ):
                    ps2 = [
                        mm2ps.tile([P, 512], F32, name=f"y2_{i}", tag=f"y2_{i}")
                        for i in range(2 * TT)
                    ]
                    for kt in range(KT_F):
                        w2t = w2p.tile([P, E], BF, tag="w2t")
                        nc.sync.dma_start(w2t, w2_d[kt])
                        for t in range(TT):
                            for n in range(E // 512):
                                nc.tensor.matmul(
                                    ps2[t * 2 + n],
                                    fT_sb[:, kt, t * P : (t + 1) * P],
                                    w2t[:, n * 512 : (n + 1) * 512],
                                    start=(kt == 0), stop=False,
                                )
                    for t in range(TT):
                        for n in range(E // 512):
                            ns = slice(n * 512, (n + 1) * 512)
                            nc.tensor.matmul(
                                ps2[t * 2 + n], ones_sb, b2_sb[0:1, ns],
                                start=False, stop=True,
                            )
                            yt = yout.tile([P, 512], F32, tag="yt")
                            nc.vector.tensor_add(yt, ps2[t * 2 + n], x2_sb[:, t, ns])
                            nc.sync.dma_start(y_view[:, t, ns], yt)

    nc.compile()
    return nc


_NC_CACHE = {}


def _get_nc(C):
    if C not in _NC_CACHE:
        _NC_CACHE[C] = _build(C)
    return _NC_CACHE[C]


def make_in_maps(inputs, C):
    """Host-side sharding + LN-gain folding. inputs values are numpy fp32."""
    TPC = C // NCORES
    KTE = E // P
    x = np.ascontiguousarray(inputs["x"].reshape(C, E).astype(np.float32))
    Wq, Wk, Wv = inputs["Wq"], inputs["Wk"], inputs["Wv"]
    Wo, bo = inputs["Wo"], inputs["bo"]
    W1, b1, W2, b2 = inputs["W1"], inputs["b1"], inputs["W2"], inputs["b2"]
    g1, bl1 = inputs["ln1_g"].astype(np.float64), inputs["ln1_b"].astype(np.float64)
    g2, bl2 = inputs["ln2_g"].astype(np.float64), inputs["ln2_b"].astype(np.float64)

    wo_h = np.ascontiguousarray(Wo.reshape(KTE, P, D).astype(bf16))
    w1_h = np.ascontiguousarray(
        (g2[:, None] * W1.astype(np.float64)).astype(np.float32)
        .reshape(KTE, P, FF).astype(bf16)
    )
    b1_eff = (b1.astype(np.float64) + bl2 @ W1.astype(np.float64)).astype(np.float32)
    b1c = np.ascontiguousarray(b1_eff.reshape(FF // P, P).T)  # (P, NFT)
    w2_h = np.ascontiguousarray(W2.reshape(FF // P, P, E).astype(bf16))
    b2r = np.ascontiguousarray(b2.reshape(1, E).astype(np.float32))
    bor = np.ascontiguousarray(bo.reshape(1, D).astype(np.float32))
    mask = np.ascontiguousarray(np.triu(np.ones((P, P), np.float32)).astype(bf16))

    in_maps = []
    for i in range(NCORES):
        h0, h1 = HPC * i, HPC * i + 1
        wq_eff = np.concatenate(
            [(g1[:, None] * Wq[h].astype(np.float64)) for h in (h0, h1)], axis=1
        ).astype(np.float32)  # (E, 128)
        wk_eff = np.concatenate(
            [(g1[:, None] * Wk[h].astype(np.float64)) for h in (h0, h1)], axis=1
        ).astype(np.float32)
        wv_eff = np.concatenate(
            [(g1[:, None] * Wv[h].astype(np.float64)) for h in (h0, h1)], axis=1
        ).astype(np.float32)
        bq = np.concatenate(
            [bl1 @ Wq[h].astype(np.float64) for h in (h0, h1)]
        ).astype(np.float32)
        bv = np.concatenate(
            [bl1 @ Wv[h].astype(np.float64) for h in (h0, h1)]
        ).astype(np.float32)
        in_maps.append(
            {
                "x": x,
                "xs": np.ascontiguousarray(x[i * TPC : (i + 1) * TPC]),
                "wq": np.ascontiguousarray(wq_eff.reshape(KTE, P, P).astype(bf16)),
                "wk": np.ascontiguousarray(wk_eff.reshape(KTE, P, P).astype(bf16)),
                "wv": np.ascontiguousarray(wv_eff.reshape(KTE, P, P).astype(bf16)),
                "bq": np.ascontiguousarray(bq.reshape(P, 1)),
                "bv": np.ascontiguousarray(bv.reshape(P, 1)),
                "wo": wo_h,
                "bo_r": bor,
                "w1": w1_h,
                "b1c": b1c,
                "w2": w2_h,
                "b2_r": b2r,
                "mask": mask,
            }
        )
    return in_maps


def run(inputs, C=4096, trace=False):
    nc = _get_nc(C)
    in_maps = make_in_maps(inputs, C)
    res = run_bass_kernel_spmd(nc, in_maps, core_ids=list(range(NCORES)), trace=trace)
    TPC = C // NCORES
    y = np.concatenate(
        [np.asarray(res.results[i]["y"]).reshape(TPC, E) for i in range(NCORES)], 0
    )
    return y.reshape(1, C, E).astype(np.float32), res


def kernel(**inputs):
    inputs = {k: np.asarray(v) for k, v in inputs.items()}
    y, _ = run(inputs, C=4096, trace=False)
    return y


# revision 7
# speedup vs baseline: 1.0720x; 1.0720x over previous
"""Trainium2 Bass kernel for a pre-norm causal-attention transformer layer.

Contract: kernel(**inputs) takes the FULL fp32 inputs of reference.setup_inputs()
and returns the FULL (1, 4096, 1024) fp32 output, distributing across 8
NeuronCores internally (heads tensor-parallel for attention, tokens
data-parallel for the output projection + FFN, one AllToAll in between).

Math notes (validated against the reference in fp64/numpy):
- LayerNorm gains are folded into the following weight matrices on the host:
  h @ W = z @ (g*W) + (ln_b @ W), where z = (x - mu) * rsig.
- The k-projection bias is dropped (softmax is shift-invariant along keys);
  the v bias is applied after normalization; the q bias rides the eviction.
- Softmax runs without max-subtraction (scores are bounded, |s| < ~3).
- Scores are built transposed (keys on partitions) so exp output feeds the
  PE directly; an appended ones-column of v yields the denominator row.
"""

import sys

sys.path.insert(0, "/opt/trn_rl_repo")

import ml_dtypes
import numpy as np

import concourse.bass as bass
from concourse import bacc, mybir, tile
from concourse.bass_utils import run_bass_kernel_spmd

F32 = mybir.dt.float32
BF = mybir.dt.bfloat16
bf16 = ml_dtypes.bfloat16

P = 128
E = 1024
NH = 16
HS = 64
D = 1024
FF = 4096
NCORES = 8
HPC = NH // NCORES  # heads per core = 2
LN_EPS = 1e-5
SCL = 1.0 / 32.0  # 1/sqrt(E)

Act = mybir.ActivationFunctionType
Alu = mybir.AluOpType


def _build(C):
    NT = C // P  # x tiles (32)
    NQC = C // 512  # q chunks (8)
    TPC = C // NCORES  # tokens per core (512)
    TT = TPC // P  # token tiles per core slice (4)
    NZG = max(1, NT // 8)  # zT groups of 8 x-tiles
    GL = NT // NZG  # x-tiles per zT group
    KT_E = E // P  # contraction tiles over E (8)
    KT_F = FF // P  # contraction tiles over FF (32)
    NFT = FF // P  # f tiles (32)

    nc = bacc.Bacc("TRN2", target_bir_lowering=False, debug=False, num_devices=NCORES)

    x_d = nc.dram_tensor("x", [C, E], F32, kind="ExternalInput")
    xs_d = nc.dram_tensor("xs", [TPC, E], F32, kind="ExternalInput")
    wq_d = nc.dram_tensor("wq", [KT_E, P, P], BF, kind="ExternalInput")
    wk_d = nc.dram_tensor("wk", [KT_E, P, P], BF, kind="ExternalInput")
    wv_d = nc.dram_tensor("wv", [KT_E, P, P], BF, kind="ExternalInput")
    bq_d = nc.dram_tensor("bq", [P, 1], F32, kind="ExternalInput")
    bv_d = nc.dram_tensor("bv", [P, 1], F32, kind="ExternalInput")
    wo_d = nc.dram_tensor("wo", [KT_E, P, D], BF, kind="ExternalInput")
    bo_d = nc.dram_tensor("bo_r", [1, D], F32, kind="ExternalInput")
    w1_d = nc.dram_tensor("w1", [KT_E, P, FF], BF, kind="ExternalInput")
    b1_d = nc.dram_tensor("b1c", [P, NFT], F32, kind="ExternalInput")
    w2_d = nc.dram_tensor("w2", [KT_F, P, E], BF, kind="ExternalInput")
    b2_d = nc.dram_tensor("b2_r", [1, E], F32, kind="ExternalInput")
    mask_d = nc.dram_tensor("mask", [P, P], BF, kind="ExternalInput")
    y_d = nc.dram_tensor("y", [TPC, E], F32, kind="ExternalOutput")
    y_view = y_d.ap().rearrange("(tc p) e -> p tc e", p=P)
    xs_view = xs_d.ap().rearrange("(tc p) e -> p tc e", p=P)

    with tile.TileContext(nc) as tc:
        with (
            tc.tile_pool(name="consts", bufs=1) as consts,
            tc.tile_pool(name="dram", bufs=1, space="DRAM") as dram,
        ):
            # ---- constants / weights resident in SBUF (~23 KB/part) ----
            wq_sb = consts.tile([P, KT_E, P], BF, tag="wq")
            wk_sb = consts.tile([P, KT_E, P], BF, tag="wk")
            wv_sb = consts.tile([P, KT_E, P], BF, tag="wv")
            nc.sync.dma_start(wq_sb, wq_d.ap().rearrange("k p m -> p k m"))
            nc.sync.dma_start(wk_sb, wk_d.ap().rearrange("k p m -> p k m"))
            nc.sync.dma_start(wv_sb, wv_d.ap().rearrange("k p m -> p k m"))
            wo_sb = consts.tile([P, KT_E, D], BF, tag="wo")
            nc.sync.dma_start(wo_sb, wo_d.ap().rearrange("k p n -> p k n"))
            bq_sb = consts.tile([P, 1], F32, tag="bq")
            bv_sb = consts.tile([P, 1], F32, tag="bv")
            nc.sync.dma_start(bq_sb, bq_d.ap())
            nc.sync.dma_start(bv_sb, bv_d.ap())
            bo_sb = consts.tile([1, D], F32, tag="bo")
            b2_sb = consts.tile([1, E], F32, tag="b2")
            nc.sync.dma_start(bo_sb, bo_d.ap())
            nc.sync.dma_start(b2_sb, b2_d.ap())
            b1_sb = consts.tile([P, NFT], F32, tag="b1")
            nc.sync.dma_start(b1_sb, b1_d.ap())
            mask_sb = consts.tile([P, P], BF, tag="mask")
            nc.sync.dma_start(mask_sb, mask_d.ap())
            eps_sb = consts.tile([P, 1], F32, tag="eps")
            nc.vector.memset(eps_sb, LN_EPS)
            ones_sb = consts.tile([1, P], F32, tag="ones")
            nc.vector.memset(ones_sb, 1.0)

            a2a_in = dram.tile([NCORES, P, TPC], BF, tag="a2a_in")
            a2a_out = dram.tile([NCORES, P, TPC], BF, tag="a2a_out")

            # ======== attention scope: qT/kT/v/outT (~32 KB/part) ========
            # chunked tiles (one per 512-token chunk) let attention start on
            # early chunks while q/k/v projections still run on later ones
            with tc.tile_pool(name="attnb", bufs=1) as attnb:
                NCH = C // 512
                qT_c = [attnb.tile([P, 512], BF, name=f"qT{c}", tag=f"qT{c}")
                        for c in range(NCH)]
                kT_c = [attnb.tile([P, 512], BF, name=f"kT{c}", tag=f"kT{c}")
                        for c in range(NCH)]
                v_c = [attnb.tile([P, 4, HPC, HS + 1], BF, name=f"v{c}", tag=f"v{c}")
                       for c in range(NCH)]
                outT_c = [attnb.tile([P, 512], BF, name=f"oc{c}", tag=f"oc{c}")
                          for c in range(NCH)]
                for c in range(NCH):
                    nc.vector.memset(v_c[c][:, :, :, HS : HS + 1], 1.0)

                # ---- phase 1: LN1 + transpose (z with E on partitions) ----
                with (
                    tc.tile_pool(name="xp", bufs=3) as xp,
                    tc.tile_pool(name="zp", bufs=3) as zp,
                    tc.tile_pool(name="stp", bufs=3) as stp,
                    tc.tile_pool(name="ztp", bufs=1) as ztp,
                    tc.tile_pool(name="qkps", bufs=2, space="PSUM") as qkps,
                    tc.tile_pool(name="vps", bufs=2, space="PSUM") as vps,
                ):
                    zT_g = [
                        ztp.tile([P, GL, KT_E, P], BF, name=f"zT{g}", tag=f"zT{g}")
                        for g in range(NZG)
                    ]
                    for t in range(NT):
                        x_sb = xp.tile([P, E], F32, tag="xt")
                        nc.sync.dma_start(x_sb, x_d[t * P : (t + 1) * P, :])
                        st = stp.tile([P, 2, 6], F32, tag="st")
                        nc.vector.bn_stats(st[:, 0, :], x_sb[:, 0:512])
                        nc.vector.bn_stats(st[:, 1, :], x_sb[:, 512:1024])
                        mv = stp.tile([P, 2], F32, tag="mv")
                        nc.vector.bn_aggr(mv, st)
                        sig = stp.tile([P, 1], F32, tag="sig")
                        nc.scalar.activation(
                            sig, mv[:, 1:2], Act.Sqrt, bias=eps_sb, scale=1.0
                        )
                        rsig = stp.tile([P, 1], F32, tag="rsig")
                        nc.vector.reciprocal(rsig, sig)
                        negb = stp.tile([P, 1], F32, tag="negb")
                        nc.vector.tensor_scalar(
                            negb, mv[:, 0:1], rsig, -1.0, Alu.mult, Alu.mult
                        )
                        z_sb = zp.tile([P, E], BF, tag="zt")
                        nc.scalar.activation(
                            z_sb, x_sb, Act.Identity, bias=negb, scale=rsig
                        )
                        nc.sync.dma_start(
                            zT_g[t // GL][:, t % GL, :, :], z_sb, transpose=True
                        )

                    # ---- phase 2: q/k/v projections (2 heads stacked, M=128) ----
                    for c in range(NCH):
                        g, cl = (c * 4) // GL, (c * 4) % GL
                        rhs = zT_g[g][:, cl : cl + 4, :, :]
                        for nm, w, dst in (("q", wq_sb, qT_c[c]), ("k", wk_sb, kT_c[c])):
                            ps = qkps.tile([P, 512], F32, tag="qk")
                            for kt in range(KT_E):
                                nc.tensor.matmul(
                                    ps, w[:, kt, :], rhs[:, :, kt, :],
                                    start=(kt == 0), stop=(kt == KT_E - 1),
                                )
                            if nm == "q":
                                nc.scalar.activation(
                                    dst[:], ps, Act.Identity, bias=bq_sb, scale=1.0
                                )
                            else:
                                nc.vector.tensor_copy(dst[:], ps)
                        for tl in range(4):
                            t = c * 4 + tl
                            ps = vps.tile([P, P], F32, tag="vt")
                            for kt in range(KT_E):
                                nc.tensor.matmul(
                                    ps, zT_g[t // GL][:, t % GL, kt, :], wv_sb[:, kt, :],
                                    start=(kt == 0), stop=(kt == KT_E - 1),
                                )
                            nc.scalar.copy(v_c[c][:, tl, 0, 0:HS], ps[:, 0:HS])
                            nc.vector.tensor_copy(v_c[c][:, tl, 1, 0:HS], ps[:, HS:P])

                # ---- phase 3: causal attention, transposed-score layout ----
                with (
                    tc.tile_pool(name="stps", bufs=3, space="PSUM") as stps,
                    tc.tile_pool(name="avps", bufs=2, space="PSUM") as avps,
                    tc.tile_pool(name="ep", bufs=4) as ep,
                    tc.tile_pool(name="nrm", bufs=3) as nrm,
                    tc.tile_pool(name="recd", bufs=2, space="DRAM") as recd,
                ):
                    for qc in range(NQC):
                        for h in range(HPC):
                            hsl = slice(h * HS, (h + 1) * HS)
                            q_rhs = qT_c[qc][hsl, :]
                            av = avps.tile([HS + 1, 512], F32, tag="av")
                            nkb = 4 * qc + 4
                            for pr in range(nkb // 2):
                                # two key-blocks share one 2-bank psum so a
                                # single wide exp amortizes ACT overhead
                                sT = stps.tile([P, 1024], F32, tag="sT")
                                ex = ep.tile([P, 1024], BF, tag="ex")
                                for half in range(2):
                                    kb = 2 * pr + half
                                    nc.tensor.matmul(
                                        sT[:, half * 512 : half * 512 + 512],
                                        kT_c[kb // 4][hsl, (kb % 4) * P : (kb % 4 + 1) * P],
                                        q_rhs, start=True, stop=True,
                                    )
                                if 2 * pr + 1 < 4 * qc:
                                    nc.scalar.activation(
                                        ex, sT, Act.Exp, bias=0.0, scale=SCL
                                    )
                                else:
                                    for half in range(2):
                                        kb = 2 * pr + half
                                        dd = kb - 4 * qc
                                        o = half * 512
                                        if dd < 0:
                                            nc.scalar.activation(
                                                ex[:, o : o + 512], sT[:, o : o + 512],
                                                Act.Exp, bias=0.0, scale=SCL,
                                            )
                                            continue
                                        if dd > 0:
                                            nc.gpsimd.memset(ex[:, o : o + dd * P], 0.0)
                                        nc.scalar.activation(
                                            ex[:, o + dd * P : o + 512],
                                            sT[:, o + dd * P : o + 512],
                                            Act.Exp, bias=0.0, scale=SCL,
                                        )
                                        nc.vector.tensor_mul(
                                            ex[:, o + dd * P : o + (dd + 1) * P],
                                            ex[:, o + dd * P : o + (dd + 1) * P],
                                            mask_sb,
                                        )
                                for half in range(2):
                                    kb = 2 * pr + half
                                    nc.tensor.matmul(
                                        av, v_c[kb // 4][:, kb % 4, h, :],
                                        ex[:, half * 512 : half * 512 + 512],
                                        start=(kb == 0), stop=(kb == nkb - 1),
                                    )
                            # normalize rows 0..63 by denominator row 64
                            rec = nrm.tile([1, 512], F32, tag="rec")
                            nc.vector.reciprocal(rec, av[HS : HS + 1, :])
                            rdr = recd.tile([1, 512], F32, tag="rdr")
                            nc.sync.dma_start(rdr, rec)
                            rd = rdr[:]
                            bc = nrm.tile([HS, 512], F32, tag="bc")
                            nc.sync.dma_start(
                                bc,
                                bass.AP(tensor=rd.tensor, offset=rd.offset,
                                        ap=[[0, HS], rd.ap[-1]]),
                            )
                            tmp = nrm.tile([HS, 512], F32, tag="tmp")
                            nc.vector.tensor_mul(tmp, av[0:HS, :], bc)
                            nc.vector.tensor_scalar(
                                outT_c[qc][hsl, :], tmp,
                                bv_sb[hsl, 0:1], None, Alu.add,
                            )
                        # ship this chunk to its a2a slot as soon as both
                        # heads are normalized
                        a2a_v3 = a2a_in[:].rearrange("j p t -> p j t")
                        if TPC >= 512:
                            nc.sync.dma_start(a2a_v3[:, qc, :], outT_c[qc][:])
                        else:
                            nj = 512 // TPC
                            nc.sync.dma_start(
                                a2a_v3[:, qc * nj : (qc + 1) * nj, :], outT_c[qc][:]
                            )

            # ======== FFN scope (attention buffers released) ========
            nc.gpsimd.collective_compute(
                "AllToAll", Alu.bypass,
                replica_groups=[list(range(NCORES))],
                ins=[a2a_in[:].opt()], outs=[a2a_out[:].opt()],
            )
            with tc.tile_pool(name="ffnb", bufs=1) as ffnb:
                oT_sb = ffnb.tile([P, KT_E, TPC], BF, tag="oT")
                nc.sync.dma_start(oT_sb, a2a_out[:].rearrange("j p t -> p j t"))
                x2_sb = ffnb.tile([P, TT, E], F32, tag="x2")
                fT_sb = ffnb.tile([P, NFT, TPC], BF, tag="fT")
                z2T_sb = ffnb.tile([P, TT, KT_E, P], BF, tag="z2T")

                with (
                    tc.tile_pool(name="w1p", bufs=1) as w1p,
                    tc.tile_pool(name="xsp", bufs=2) as xsp,
                    tc.tile_pool(name="st2p", bufs=2) as st2p,
                    tc.tile_pool(name="z2p", bufs=2) as z2p,
                    tc.tile_pool(name="wops", bufs=2, space="PSUM") as wops,
                    tc.tile_pool(name="mm1ps", bufs=3, space="PSUM") as mm1ps,
                ):
                    w1_sb = [
                        w1p.tile([P, FF], BF, name=f"w1_{k}", tag=f"w1_{k}")
                        for k in range(KT_E)
                    ]
                    for kt in range(KT_E):
                        nc.sync.dma_start(w1_sb[kt], w1_d[kt])

                    # ---- phase 4: Wo projection + residual + LN2 ----
                    for t in range(TT):
                        xs_t = xsp.tile([P, E], F32, tag="xst")
                        nc.sync.dma_start(xs_t, xs_view[:, t, :])
                        for n in range(E // 512):
                            ns = slice(n * 512, (n + 1) * 512)
                            ps = wops.tile([P, 512], F32, tag="wo")
                            for kt in range(KT_E):
                                nc.tensor.matmul(
                                    ps, oT_sb[:, kt, t * P : (t + 1) * P],
                                    wo_sb[:, kt, ns],
                                    start=(kt == 0), stop=False,
                                )
                            nc.tensor.matmul(
                                ps, ones_sb, bo_sb[0:1, ns], start=False, stop=True
                            )
                            nc.vector.tensor_add(x2_sb[:, t, ns], ps, xs_t[:, ns])

                        st = st2p.tile([P, 2, 6], F32, tag="st2")
                        nc.vector.bn_stats(st[:, 0, :], x2_sb[:, t, 0:512])
                        nc.vector.bn_stats(st[:, 1, :], x2_sb[:, t, 512:1024])
                        mv = st2p.tile([P, 2], F32, tag="mv2")
                        nc.vector.bn_aggr(mv, st)
                        sig = st2p.tile([P, 1], F32, tag="sig2")
                        nc.scalar.activation(
                            sig, mv[:, 1:2], Act.Sqrt, bias=eps_sb, scale=1.0
                        )
                        rsig = st2p.tile([P, 1], F32, tag="rsig2")
                        nc.vector.reciprocal(rsig, sig)
                        negb = st2p.tile([P, 1], F32, tag="negb2")
                        nc.vector.tensor_scalar(
                            negb, mv[:, 0:1], rsig, -1.0, Alu.mult, Alu.mult
                        )
                        z2 = z2p.tile([P, E], BF, tag="z2")
                        nc.scalar.activation(
                            z2, x2_sb[:, t, :], Act.Identity, bias=negb, scale=rsig
                        )
                        nc.sync.dma_start(z2T_sb[:, t, :, :], z2, transpose=True)

                    # ---- phase 5a: fT = relu(W1.T @ z2T + b1) ----
                    for ft in range(NFT):
                        ps = mm1ps.tile([P, TPC], F32, tag="mm1")
                        for kt in range(KT_E):
                            nc.tensor.matmul(
                                ps, w1_sb[kt][:, ft * P : (ft + 1) * P],
                                z2T_sb[:, :, kt, :],
                                start=(kt == 0), stop=(kt == KT_E - 1),
                            )
                        nc.scalar.activation(
                            fT_sb[:, ft, :], ps, Act.Relu,
                            bias=b1_sb[:, ft : ft + 1], scale=1.0,
                        )

                # ---- phase 5b: y = fT.T @ W2 + b2 + x2 ----
                with (
                    tc.tile_pool(name="mm2ps", bufs=1, space="PSUM") as mm2ps,
                    tc.tile_pool(name="w2p", bufs=3) as w2p,
                    tc.tile_pool(name="yout", bufs=3) as yout,
                ):
                    ps2 = [
                        mm2ps.tile([P, 512], F32, name=f"y2_{i}", tag=f"y2_{i}")
                        for i in range(2 * TT)
                    ]
                    for kt in range(KT_F):
                        w2t = w2p.tile([P, E], BF, tag="w2t")
                        nc.sync.dma_start(w2t, w2_d[kt])
                        for t in range(TT):
                            for n in range(E // 512):
                                nc.tensor.matmul(
                                    ps2[t * 2 + n],
                                    fT_sb[:, kt, t * P : (t + 1) * P],
                                    w2t[:, n * 512 : (n + 1) * 512],
                                    start=(kt == 0), stop=False,
                                )
                    for t in range(TT):
                        for n in range(E // 512):
                            ns = slice(n * 512, (n + 1) * 512)
                            nc.tensor.matmul(
                                ps2[t * 2 + n], ones_sb, b2_sb[0:1, ns],
                                start=False, stop=True,
                            )
                            yt = yout.tile([P, 512], F32, tag="yt")
                            nc.vector.tensor_add(yt, ps2[t * 2 + n], x2_sb[:, t, ns])
                            nc.sync.dma_start(y_view[:, t, ns], yt)

    nc.compile()
    return nc


_NC_CACHE = {}


def _get_nc(C):
    if C not in _NC_CACHE:
        _NC_CACHE[C] = _build(C)
    return _NC_CACHE[C]


def make_in_maps(inputs, C):
    """Host-side sharding + LN-gain folding. inputs values are numpy fp32."""
    TPC = C // NCORES
    KTE = E // P
    x = np.ascontiguousarray(inputs["x"].reshape(C, E).astype(np.float32))
    Wq, Wk, Wv = inputs["Wq"], inputs["Wk"], inputs["Wv"]
    Wo, bo = inputs["Wo"], inputs["bo"]
    W1, b1, W2, b2 = inputs["W1"], inputs["b1"], inputs["W2"], inputs["b2"]
    g1, bl1 = inputs["ln1_g"].astype(np.float64), inputs["ln1_b"].astype(np.float64)
    g2, bl2 = inputs["ln2_g"].astype(np.float64), inputs["ln2_b"].astype(np.float64)

    wo_h = np.ascontiguousarray(Wo.reshape(KTE, P, D).astype(bf16))
    w1_h = np.ascontiguousarray(
        (g2[:, None] * W1.astype(np.float64)).astype(np.float32)
        .reshape(KTE, P, FF).astype(bf16)
    )
    b1_eff = (b1.astype(np.float64) + bl2 @ W1.astype(np.float64)).astype(np.float32)
    b1c = np.ascontiguousarray(b1_eff.reshape(FF // P, P).T)  # (P, NFT)
    w2_h = np.ascontiguousarray(W2.reshape(FF // P, P, E).astype(bf16))
    b2r = np.ascontiguousarray(b2.reshape(1, E).astype(np.float32))
    bor = np.ascontiguousarray(bo.reshape(1, D).astype(np.float32))
    mask = np.ascontiguousarray(np.triu(np.ones((P, P), np.float32)).astype(bf16))

    in_maps = []
    for i in range(NCORES):
        h0, h1 = HPC * i, HPC * i + 1
        wq_eff = np.concatenate(
            [(g1[:, None] * Wq[h].astype(np.float64)) for h in (h0, h1)], axis=1
        ).astype(np.float32)  # (E, 128)
        wk_eff = np.concatenate(
            [(g1[:, None] * Wk[h].astype(np.float64)) for h in (h0, h1)], axis=1
        ).astype(np.float32)
        wv_eff = np.concatenate(
            [(g1[:, None] * Wv[h].astype(np.float64)) for h in (h0, h1)], axis=1
        ).astype(np.float32)
        bq = np.concatenate(
            [bl1 @ Wq[h].astype(np.float64) for h in (h0, h1)]
        ).astype(np.float32)
        bv = np.concatenate(
            [bl1 @ Wv[h].astype(np.float64) for h in (h0, h1)]
        ).astype(np.float32)
        in_maps.append(
            {
                "x": x,
                "xs": np.ascontiguousarray(x[i * TPC : (i + 1) * TPC]),
                "wq": np.ascontiguousarray(wq_eff.reshape(KTE, P, P).astype(bf16)),
                "wk": np.ascontiguousarray(wk_eff.reshape(KTE, P, P).astype(bf16)),
                "wv": np.ascontiguousarray(wv_eff.reshape(KTE, P, P).astype(bf16)),
                "bq": np.ascontiguousarray(bq.reshape(P, 1)),
                "bv": np.ascontiguousarray(bv.reshape(P, 1)),
                "wo": wo_h,
                "bo_r": bor,
                "w1": w1_h,
                "b1c": b1c,
                "w2": w2_h,
                "b2_r": b2r,
                "mask": mask,
            }
        )
    return in_maps


def run(inputs, C=4096, trace=False):
    nc = _get_nc(C)
    in_maps = make_in_maps(inputs, C)
    res = run_bass_kernel_spmd(nc, in_maps, core_ids=list(range(NCORES)), trace=trace)
    TPC = C // NCORES
    y = np.concatenate(
        [np.asarray(res.results[i]["y"]).reshape(TPC, E) for i in range(NCORES)], 0
    )
    return y.reshape(1, C, E).astype(np.float32), res


def kernel(**inputs):
    inputs = {k: np.asarray(v) for k, v in inputs.items()}
    y, _ = run(inputs, C=4096, trace=False)
    return y


# revision 9
# speedup vs baseline: 1.1489x; 1.0717x over previous
"""Trainium2 Bass kernel for a pre-norm causal-attention transformer layer.

Contract: kernel(**inputs) takes the FULL fp32 inputs of reference.setup_inputs()
and returns the FULL (1, 4096, 1024) fp32 output, distributing across 8
NeuronCores internally (heads tensor-parallel for attention, tokens
data-parallel for the output projection + FFN, one AllToAll in between).

Math notes (validated against the reference in fp64/numpy):
- LayerNorm gains are folded into the following weight matrices on the host:
  h @ W = z @ (g*W) + (ln_b @ W), where z = (x - mu) * rsig.
- The k-projection bias is dropped (softmax is shift-invariant along keys);
  the v bias is applied after normalization; the q bias rides the eviction.
- Softmax runs without max-subtraction (scores are bounded, |s| < ~3).
- Scores are built transposed (keys on partitions) so exp output feeds the
  PE directly; an appended ones-column of v yields the denominator row.
"""

import sys

sys.path.insert(0, "/opt/trn_rl_repo")

import ml_dtypes
import numpy as np

import concourse.bass as bass
from concourse import bacc, mybir, tile
from concourse.bass_utils import run_bass_kernel_spmd

F32 = mybir.dt.float32
BF = mybir.dt.bfloat16
bf16 = ml_dtypes.bfloat16

P = 128
E = 1024
NH = 16
HS = 64
D = 1024
FF = 4096
NCORES = 8
HPC = NH // NCORES  # heads per core = 2
LN_EPS = 1e-5
SCL = 1.0 / 32.0  # 1/sqrt(E)

Act = mybir.ActivationFunctionType
Alu = mybir.AluOpType


def _build(C):
    NT = C // P  # x tiles (32)
    NQC = C // 512  # q chunks (8)
    TPC = C // NCORES  # tokens per core (512)
    TT = TPC // P  # token tiles per core slice (4)
    NZG = max(1, NT // 8)  # zT groups of 8 x-tiles
    GL = NT // NZG  # x-tiles per zT group
    KT_E = E // P  # contraction tiles over E (8)
    KT_F = FF // P  # contraction tiles over FF (32)
    NFT = FF // P  # f tiles (32)

    nc = bacc.Bacc("TRN2", target_bir_lowering=False, debug=False, num_devices=NCORES)

    x_d = nc.dram_tensor("x", [C, E], F32, kind="ExternalInput")
    xs_d = nc.dram_tensor("xs", [TPC, E], F32, kind="ExternalInput")
    wq_d = nc.dram_tensor("wq", [KT_E, P, P], BF, kind="ExternalInput")
    wk_d = nc.dram_tensor("wk", [KT_E, P, P], BF, kind="ExternalInput")
    wv_d = nc.dram_tensor("wv", [KT_E, P, P], BF, kind="ExternalInput")
    bq_d = nc.dram_tensor("bq", [P, 1], F32, kind="ExternalInput")
    bv_d = nc.dram_tensor("bv", [P, 1], F32, kind="ExternalInput")
    wo_d = nc.dram_tensor("wo", [KT_E, P, D], BF, kind="ExternalInput")
    bo_d = nc.dram_tensor("bo_r", [1, D], F32, kind="ExternalInput")
    w1_d = nc.dram_tensor("w1", [KT_E, P, FF], BF, kind="ExternalInput")
    b1_d = nc.dram_tensor("b1c", [P, NFT], F32, kind="ExternalInput")
    w2_d = nc.dram_tensor("w2", [KT_F, P, E], BF, kind="ExternalInput")
    b2_d = nc.dram_tensor("b2_r", [1, E], F32, kind="ExternalInput")
    mask_d = nc.dram_tensor("mask", [P, P], BF, kind="ExternalInput")
    maskz_d = nc.dram_tensor("maskz", [4, P, 512], BF, kind="ExternalInput")
    y_d = nc.dram_tensor("y", [TPC, E], F32, kind="ExternalOutput")
    y_view = y_d.ap().rearrange("(tc p) e -> p tc e", p=P)
    xs_view = xs_d.ap().rearrange("(tc p) e -> p tc e", p=P)

    with tile.TileContext(nc) as tc:
        with (
            tc.tile_pool(name="consts", bufs=1) as consts,
            tc.tile_pool(name="dram", bufs=1, space="DRAM") as dram,
        ):
            # ---- constants / weights resident in SBUF (~23 KB/part) ----
            wq_sb = consts.tile([P, KT_E, P], BF, tag="wq")
            wk_sb = consts.tile([P, KT_E, P], BF, tag="wk")
            wv_sb = consts.tile([P, KT_E, P], BF, tag="wv")
            nc.sync.dma_start(wq_sb, wq_d.ap().rearrange("k p m -> p k m"))
            nc.sync.dma_start(wk_sb, wk_d.ap().rearrange("k p m -> p k m"))
            nc.sync.dma_start(wv_sb, wv_d.ap().rearrange("k p m -> p k m"))
            wo_sb = consts.tile([P, KT_E, D], BF, tag="wo")
            nc.sync.dma_start(wo_sb, wo_d.ap().rearrange("k p n -> p k n"))
            bq_sb = consts.tile([P, 1], F32, tag="bq")
            bv_sb = consts.tile([P, 1], F32, tag="bv")
            nc.sync.dma_start(bq_sb, bq_d.ap())
            nc.sync.dma_start(bv_sb, bv_d.ap())
            bo_sb = consts.tile([1, D], F32, tag="bo")
            b2_sb = consts.tile([1, E], F32, tag="b2")
            nc.sync.dma_start(bo_sb, bo_d.ap())
            nc.sync.dma_start(b2_sb, b2_d.ap())
            b1_sb = consts.tile([P, NFT], F32, tag="b1")
            nc.sync.dma_start(b1_sb, b1_d.ap())
            mask_sb = consts.tile([P, P], BF, tag="mask")
            nc.sync.dma_start(mask_sb, mask_d.ap())
            maskz_sb = consts.tile([P, 4, 512], BF, tag="maskz")
            nc.sync.dma_start(maskz_sb, maskz_d.ap().rearrange("d p t -> p d t"))
            eps_sb = consts.tile([P, 1], F32, tag="eps")
            nc.vector.memset(eps_sb, LN_EPS)
            ones_sb = consts.tile([1, P], F32, tag="ones")
            nc.vector.memset(ones_sb, 1.0)

            a2a_in = dram.tile([NCORES, P, TPC], BF, tag="a2a_in")
            a2a_out = dram.tile([NCORES, P, TPC], BF, tag="a2a_out")

            # ======== attention scope: qT/kT/v/outT (~32 KB/part) ========
            # chunked tiles (one per 512-token chunk) let attention start on
            # early chunks while q/k/v projections still run on later ones
            with tc.tile_pool(name="attnb", bufs=1) as attnb:
                NCH = C // 512
                qT_c = [attnb.tile([P, 512], BF, name=f"qT{c}", tag=f"qT{c}")
                        for c in range(NCH)]
                kT_c = [attnb.tile([P, 512], BF, name=f"kT{c}", tag=f"kT{c}")
                        for c in range(NCH)]
                v_c = [attnb.tile([P, 4, HPC, HS + 1], BF, name=f"v{c}", tag=f"v{c}")
                       for c in range(NCH)]
                outT_c = [attnb.tile([P, 512], BF, name=f"oc{c}", tag=f"oc{c}")
                          for c in range(NCH)]
                for c in range(NCH):
                    nc.vector.memset(v_c[c][:, :, :, HS : HS + 1], 1.0)

                # ---- phase 1: LN1 + transpose (z with E on partitions) ----
                with (
                    tc.tile_pool(name="xp", bufs=3) as xp,
                    tc.tile_pool(name="zp", bufs=3) as zp,
                    tc.tile_pool(name="stp", bufs=3) as stp,
                    tc.tile_pool(name="ztp", bufs=1) as ztp,
                    tc.tile_pool(name="qkps", bufs=2, space="PSUM") as qkps,
                    tc.tile_pool(name="vps", bufs=2, space="PSUM") as vps,
                ):
                    zT_g = [
                        ztp.tile([P, GL, KT_E, P], BF, name=f"zT{g}", tag=f"zT{g}")
                        for g in range(NZG)
                    ]
                    for t in range(NT):
                        x_sb = xp.tile([P, E], F32, tag="xt")
                        nc.sync.dma_start(x_sb, x_d[t * P : (t + 1) * P, :])
                        st = stp.tile([P, 2, 6], F32, tag="st")
                        nc.vector.bn_stats(st[:, 0, :], x_sb[:, 0:512])
                        nc.vector.bn_stats(st[:, 1, :], x_sb[:, 512:1024])
                        mv = stp.tile([P, 2], F32, tag="mv")
                        nc.vector.bn_aggr(mv, st)
                        sig = stp.tile([P, 1], F32, tag="sig")
                        nc.scalar.activation(
                            sig, mv[:, 1:2], Act.Sqrt, bias=eps_sb, scale=1.0
                        )
                        rsig = stp.tile([P, 1], F32, tag="rsig")
                        nc.vector.reciprocal(rsig, sig)
                        negb = stp.tile([P, 1], F32, tag="negb")
                        nc.vector.tensor_scalar(
                            negb, mv[:, 0:1], rsig, -1.0, Alu.mult, Alu.mult
                        )
                        z_sb = zp.tile([P, E], BF, tag="zt")
                        nc.scalar.activation(
                            z_sb, x_sb, Act.Identity, bias=negb, scale=rsig
                        )
                        nc.sync.dma_start(
                            zT_g[t // GL][:, t % GL, :, :], z_sb, transpose=True
                        )

                    # ---- phase 2: q/k/v projections (2 heads stacked, M=128) ----
                    for c in range(NCH):
                        g, cl = (c * 4) // GL, (c * 4) % GL
                        rhs = zT_g[g][:, cl : cl + 4, :, :]
                        for nm, w, dst in (("q", wq_sb, qT_c[c]), ("k", wk_sb, kT_c[c])):
                            ps = qkps.tile([P, 512], F32, tag="qk")
                            for kt in range(KT_E):
                                nc.tensor.matmul(
                                    ps, w[:, kt, :], rhs[:, :, kt, :],
                                    start=(kt == 0), stop=(kt == KT_E - 1),
                                )
                            if nm == "q":
                                nc.scalar.activation(
                                    dst[:], ps, Act.Identity, bias=bq_sb, scale=1.0
                                )
                            else:
                                nc.vector.tensor_copy(dst[:], ps)
                        for tl in range(4):
                            t = c * 4 + tl
                            ps = vps.tile([P, P], F32, tag="vt")
                            for kt in range(KT_E):
                                nc.tensor.matmul(
                                    ps, zT_g[t // GL][:, t % GL, kt, :], wv_sb[:, kt, :],
                                    start=(kt == 0), stop=(kt == KT_E - 1),
                                )
                            nc.scalar.copy(v_c[c][:, tl, 0, 0:HS], ps[:, 0:HS])
                            nc.vector.tensor_copy(v_c[c][:, tl, 1, 0:HS], ps[:, HS:P])

                # ---- phase 3: causal attention, transposed-score layout ----
                with (
                    tc.tile_pool(name="stps", bufs=3, space="PSUM") as stps,
                    tc.tile_pool(name="avps", bufs=2, space="PSUM") as avps,
                    tc.tile_pool(name="ep", bufs=6) as ep,
                    tc.tile_pool(name="nrm", bufs=3) as nrm,
                    tc.tile_pool(name="recd", bufs=2, space="DRAM") as recd,
                ):
                    for qc in range(NQC):
                        for h in range(HPC):
                            hsl = slice(h * HS, (h + 1) * HS)
                            q_rhs = qT_c[qc][hsl, :]
                            av = avps.tile([HS + 1, 512], F32, tag="av")
                            nkb = 4 * qc + 4
                            for pr in range(nkb // 2):
                                # two key-blocks share one 2-bank psum so a
                                # single wide exp amortizes ACT overhead
                                sT = stps.tile([P, 1024], F32, tag="sT")
                                ex = ep.tile([P, 1024], BF, tag="ex")
                                for half in range(2):
                                    kb = 2 * pr + half
                                    nc.tensor.matmul(
                                        sT[:, half * 512 : half * 512 + 512],
                                        kT_c[kb // 4][hsl, (kb % 4) * P : (kb % 4 + 1) * P],
                                        q_rhs, start=True, stop=True,
                                    )
                                nc.scalar.activation(
                                    ex, sT, Act.Exp, bias=0.0, scale=SCL
                                )
                                for half in range(2):
                                    kb = 2 * pr + half
                                    dd = kb - 4 * qc
                                    if dd >= 0:
                                        o = half * 512
                                        nc.vector.tensor_mul(
                                            ex[:, o : o + 512], ex[:, o : o + 512],
                                            maskz_sb[:, dd, :],
                                        )
                                for half in range(2):
                                    kb = 2 * pr + half
                                    nc.tensor.matmul(
                                        av, v_c[kb // 4][:, kb % 4, h, :],
                                        ex[:, half * 512 : half * 512 + 512],
                                        start=(kb == 0), stop=(kb == nkb - 1),
                                    )
                            # normalize rows 0..63 by denominator row 64;
                            # reciprocal is reshaped to 128 partitions (a
                            # (1,512) reciprocal runs ~1.9us on one DVE lane)
                            den = nrm.tile([1, 512], F32, tag="den")
                            nc.vector.tensor_copy(den, av[HS : HS + 1, :])
                            rdr = recd.tile([1, 512], F32, tag="rdr")
                            nc.sync.dma_start(rdr, den)
                            rd = rdr[:]
                            scat = nrm.tile([P, 4], F32, tag="scat")
                            nc.sync.dma_start(
                                scat,
                                bass.AP(tensor=rd.tensor, offset=rd.offset,
                                        ap=[[4, P], [1, 4]]),
                            )
                            rec2 = nrm.tile([P, 4], F32, tag="rec2")
                            nc.vector.reciprocal(rec2, scat)
                            rdr2 = recd.tile([1, 512], F32, tag="rdr2")
                            rd2 = rdr2[:]
                            nc.sync.dma_start(
                                bass.AP(tensor=rd2.tensor, offset=rd2.offset,
                                        ap=[[4, P], [1, 4]]),
                                rec2,
                            )
                            bc = nrm.tile([HS, 512], F32, tag="bc")
                            nc.sync.dma_start(
                                bc,
                                bass.AP(tensor=rd2.tensor, offset=rd2.offset,
                                        ap=[[0, HS], rd2.ap[-1]]),
                            )
                            tmp = nrm.tile([HS, 512], F32, tag="tmp")
                            nc.vector.tensor_mul(tmp, av[0:HS, :], bc)
                            nc.vector.tensor_scalar(
                                outT_c[qc][hsl, :], tmp,
                                bv_sb[hsl, 0:1], None, Alu.add,
                            )
                        # ship this chunk to its a2a slot as soon as both
                        # heads are normalized
                        a2a_v3 = a2a_in[:].rearrange("j p t -> p j t")
                        if TPC >= 512:
                            nc.sync.dma_start(a2a_v3[:, qc, :], outT_c[qc][:])
                        else:
                            nj = 512 // TPC
                            nc.sync.dma_start(
                                a2a_v3[:, qc * nj : (qc + 1) * nj, :], outT_c[qc][:]
                            )

            # ======== FFN scope (attention buffers released) ========
            nc.gpsimd.collective_compute(
                "AllToAll", Alu.bypass,
                replica_groups=[list(range(NCORES))],
                ins=[a2a_in[:].opt()], outs=[a2a_out[:].opt()],
            )
            with tc.tile_pool(name="ffnb", bufs=1) as ffnb:
                oT_sb = ffnb.tile([P, KT_E, TPC], BF, tag="oT")
                nc.sync.dma_start(oT_sb, a2a_out[:].rearrange("j p t -> p j t"))
                x2_sb = ffnb.tile([P, TT, E], F32, tag="x2")
                fT_sb = ffnb.tile([P, NFT, TPC], BF, tag="fT")
                z2T_sb = ffnb.tile([P, TT, KT_E, P], BF, tag="z2T")

                with (
                    tc.tile_pool(name="w1p", bufs=1) as w1p,
                    tc.tile_pool(name="xsp", bufs=2) as xsp,
                    tc.tile_pool(name="st2p", bufs=2) as st2p,
                    tc.tile_pool(name="z2p", bufs=2) as z2p,
                    tc.tile_pool(name="wops", bufs=2, space="PSUM") as wops,
                    tc.tile_pool(name="mm1ps", bufs=3, space="PSUM") as mm1ps,
                ):
                    w1_sb = [
                        w1p.tile([P, FF], BF, name=f"w1_{k}", tag=f"w1_{k}")
                        for k in range(KT_E)
                    ]
                    for kt in range(KT_E):
                        nc.sync.dma_start(w1_sb[kt], w1_d[kt])

                    # ---- phase 4: Wo projection + residual + LN2 ----
                    for t in range(TT):
                        xs_t = xsp.tile([P, E], F32, tag="xst")
                        nc.sync.dma_start(xs_t, xs_view[:, t, :])
                        for n in range(E // 512):
                            ns = slice(n * 512, (n + 1) * 512)
                            ps = wops.tile([P, 512], F32, tag="wo")
                            for kt in range(KT_E):
                                nc.tensor.matmul(
                                    ps, oT_sb[:, kt, t * P : (t + 1) * P],
                                    wo_sb[:, kt, ns],
                                    start=(kt == 0), stop=False,
                                )
                            nc.tensor.matmul(
                                ps, ones_sb, bo_sb[0:1, ns], start=False, stop=True
                            )
                            nc.vector.tensor_add(x2_sb[:, t, ns], ps, xs_t[:, ns])

                        st = st2p.tile([P, 2, 6], F32, tag="st2")
                        nc.vector.bn_stats(st[:, 0, :], x2_sb[:, t, 0:512])
                        nc.vector.bn_stats(st[:, 1, :], x2_sb[:, t, 512:1024])
                        mv = st2p.tile([P, 2], F32, tag="mv2")
                        nc.vector.bn_aggr(mv, st)
                        sig = st2p.tile([P, 1], F32, tag="sig2")
                        nc.scalar.activation(
                            sig, mv[:, 1:2], Act.Sqrt, bias=eps_sb, scale=1.0
                        )
                        rsig = st2p.tile([P, 1], F32, tag="rsig2")
                        nc.vector.reciprocal(rsig, sig)
                        negb = st2p.tile([P, 1], F32, tag="negb2")
                        nc.vector.tensor_scalar(
                            negb, mv[:, 0:1], rsig, -1.0, Alu.mult, Alu.mult
                        )
                        z2 = z2p.tile([P, E], BF, tag="z2")
                        nc.scalar.activation(
                            z2, x2_sb[:, t, :], Act.Identity, bias=negb, scale=rsig
                        )
                        nc.sync.dma_start(z2T_sb[:, t, :, :], z2, transpose=True)

                    # ---- phase 5a: fT = relu(W1.T @ z2T + b1) ----
                    for ft in range(NFT):
                        ps = mm1ps.tile([P, TPC], F32, tag="mm1")
                        for kt in range(KT_E):
                            nc.tensor.matmul(
                                ps, w1_sb[kt][:, ft * P : (ft + 1) * P],
                                z2T_sb[:, :, kt, :],
                                start=(kt == 0), stop=(kt == KT_E - 1),
                            )
                        nc.scalar.activation(
                            fT_sb[:, ft, :], ps, Act.Relu,
                            bias=b1_sb[:, ft : ft + 1], scale=1.0,
                        )

                # ---- phase 5b: y = fT.T @ W2 + b2 + x2 ----
                with (
                    tc.tile_pool(name="mm2ps", bufs=1, space="PSUM") as mm2ps,
                    tc.tile_pool(name="w2p", bufs=3) as w2p,
                    tc.tile_pool(name="yout", bufs=3) as yout,
                ):
                    ps2 = [
                        mm2ps.tile([P, 512], F32, name=f"y2_{i}", tag=f"y2_{i}")
                        for i in range(2 * TT)
                    ]
                    for kt in range(KT_F):
                        w2t = w2p.tile([P, E], BF, tag="w2t")
                        nc.sync.dma_start(w2t, w2_d[kt])
                        for t in range(TT):
                            for n in range(E // 512):
                                nc.tensor.matmul(
                                    ps2[t * 2 + n],
                                    fT_sb[:, kt, t * P : (t + 1) * P],
                                    w2t[:, n * 512 : (n + 1) * 512],
                                    start=(kt == 0), stop=False,
                                )
                    for t in range(TT):
                        for n in range(E // 512):
                            ns = slice(n * 512, (n + 1) * 512)
                            nc.tensor.matmul(
                                ps2[t * 2 + n], ones_sb, b2_sb[0:1, ns],
                                start=False, stop=True,
                            )
                            yt = yout.tile([P, 512], F32, tag="yt")
                            nc.vector.tensor_add(yt, ps2[t * 2 + n], x2_sb[:, t, ns])
                            nc.sync.dma_start(y_view[:, t, ns], yt)

    nc.compile()
    return nc


_NC_CACHE = {}


def _get_nc(C):
    if C not in _NC_CACHE:
        _NC_CACHE[C] = _build(C)
    return _NC_CACHE[C]


def make_in_maps(inputs, C):
    """Host-side sharding + LN-gain folding. inputs values are numpy fp32."""
    TPC = C // NCORES
    KTE = E // P
    x = np.ascontiguousarray(inputs["x"].reshape(C, E).astype(np.float32))
    Wq, Wk, Wv = inputs["Wq"], inputs["Wk"], inputs["Wv"]
    Wo, bo = inputs["Wo"], inputs["bo"]
    W1, b1, W2, b2 = inputs["W1"], inputs["b1"], inputs["W2"], inputs["b2"]
    g1, bl1 = inputs["ln1_g"].astype(np.float64), inputs["ln1_b"].astype(np.float64)
    g2, bl2 = inputs["ln2_g"].astype(np.float64), inputs["ln2_b"].astype(np.float64)

    wo_h = np.ascontiguousarray(Wo.reshape(KTE, P, D).astype(bf16))
    w1_h = np.ascontiguousarray(
        (g2[:, None] * W1.astype(np.float64)).astype(np.float32)
        .reshape(KTE, P, FF).astype(bf16)
    )
    b1_eff = (b1.astype(np.float64) + bl2 @ W1.astype(np.float64)).astype(np.float32)
    b1c = np.ascontiguousarray(b1_eff.reshape(FF // P, P).T)  # (P, NFT)
    w2_h = np.ascontiguousarray(W2.reshape(FF // P, P, E).astype(bf16))
    b2r = np.ascontiguousarray(b2.reshape(1, E).astype(np.float32))
    bor = np.ascontiguousarray(bo.reshape(1, D).astype(np.float32))
    mask = np.ascontiguousarray(np.triu(np.ones((P, P), np.float32)).astype(bf16))
    tri = np.triu(np.ones((P, P), np.float32))
    maskz = np.zeros((4, P, 512), np.float32)
    for dd in range(4):
        maskz[dd, :, dd * P : (dd + 1) * P] = tri
        maskz[dd, :, (dd + 1) * P :] = 1.0
    maskz = np.ascontiguousarray(maskz.astype(bf16))

    in_maps = []
    for i in range(NCORES):
        h0, h1 = HPC * i, HPC * i + 1
        wq_eff = np.concatenate(
            [(g1[:, None] * Wq[h].astype(np.float64)) for h in (h0, h1)], axis=1
        ).astype(np.float32)  # (E, 128)
        wk_eff = np.concatenate(
            [(g1[:, None] * Wk[h].astype(np.float64)) for h in (h0, h1)], axis=1
        ).astype(np.float32)
        wv_eff = np.concatenate(
            [(g1[:, None] * Wv[h].astype(np.float64)) for h in (h0, h1)], axis=1
        ).astype(np.float32)
        bq = np.concatenate(
            [bl1 @ Wq[h].astype(np.float64) for h in (h0, h1)]
        ).astype(np.float32)
        bv = np.concatenate(
            [bl1 @ Wv[h].astype(np.float64) for h in (h0, h1)]
        ).astype(np.float32)
        in_maps.append(
            {
                "x": x,
                "xs": np.ascontiguousarray(x[i * TPC : (i + 1) * TPC]),
                "wq": np.ascontiguousarray(wq_eff.reshape(KTE, P, P).astype(bf16)),
                "wk": np.ascontiguousarray(wk_eff.reshape(KTE, P, P).astype(bf16)),
                "wv": np.ascontiguousarray(wv_eff.reshape(KTE, P, P).astype(bf16)),
                "bq": np.ascontiguousarray(bq.reshape(P, 1)),
                "bv": np.ascontiguousarray(bv.reshape(P, 1)),
                "wo": wo_h,
                "bo_r": bor,
                "w1": w1_h,
                "b1c": b1c,
                "w2": w2_h,
                "b2_r": b2r,
                "mask": mask,
                "maskz": maskz,
            }
        )
    return in_maps


def run(inputs, C=4096, trace=False):
    nc = _get_nc(C)
    in_maps = make_in_maps(inputs, C)
    res = run_bass_kernel_spmd(nc, in_maps, core_ids=list(range(NCORES)), trace=trace)
    TPC = C // NCORES
    y = np.concatenate(
        [np.asarray(res.results[i]["y"]).reshape(TPC, E) for i in range(NCORES)], 0
    )
    return y.reshape(1, C, E).astype(np.float32), res


def kernel(**inputs):
    inputs = {k: np.asarray(v) for k, v in inputs.items()}
    y, _ = run(inputs, C=4096, trace=False)
    return y
